# revision 1
# baseline (speedup 1.0000x reference)
"""Trainium2 Bass kernel for nn_BehlerG2 (Behler-style angular symmetry functions).

Strategy (v3 — local_scatter + fill-forward, no per-triple gather):
- 8 cores; core c handles batch b = c // 2, atom half h = c % 2 (128 atoms/core,
  one atom per SBUF partition).
- Host compacts each atom's triple list by mask, pads to Tp with a dummy
  entry 256 whose coords are 1e4 (beyond cutoff -> exact 0 contribution).
- Each atom's slots are ordered k-sorted (canonical). Neighbor fields
  (x, y, z, z_num as f32) are materialized per-atom WITHOUT any per-slot
  gather: a GPSIMD local_scatter places each table entry's f32 value (as an
  interleaved lo/hi int16 pair) at the FIRST slot of its run (per-partition
  indices; data = the shared 258-entry table). DVE jump-fill (log rounds)
  expands runs. The j-side is built in j-sorted order the same way, then one
  local_scatter per field-pair permutes it into canonical order (exact f32
  bit moves). Total: 6 local_scatter instrs (~5us each) vs 258us ap_gather.
- The 32 (eta, zeta) pair reductions run in log space:
      X[e,z,t] = -eta_e*S3_t + zeta_z*ln(1-cos theta)_t + ln(BASE_t)
  formed by 8 DVE scalar_tensor_tensor ops (bunched over z) with ACT Exp +
  accum_out doing both the exp and the sum over t.
- All math fp32; mirrors the reference (incl. the +1e-12 epsilon and the
  squared-epsilon-ed-sqrt r^2 quirk for degenerate triples).
"""

import sys

if "/opt/trn_rl_repo" not in sys.path:
    sys.path.insert(0, "/opt/trn_rl_repo")

import numpy as np

import concourse.bacc as bacc
import concourse.mybir as mybir
import concourse.tile as tile
from concourse.alu_op_type import AluOpType as alu
from concourse.bass_utils import run_bass_kernel_spmd

f32 = mybir.dt.float32
i16 = mybir.dt.int16

B, A, T = 4, 256, 512
NCORES = 8
P = 128          # atoms per core == partitions
NDUM = 16        # dummy entries (padding slots round-robin -> short fill runs)
NE = 256 + NDUM  # table entries
ZETAS = [1.0, 2.0, 4.0, 8.0]
CUTOFF = 6.0
PI = float(np.pi)
FAR = 1.0e4      # dummy-entry coordinate (beyond cutoff)

AF = mybir.ActivationFunctionType

_ROUNDS = 4      # jump-fill rounds; set by _prepare_host from max run length


def _build_program(Tp: int, etas: np.ndarray):
    rounds = _ROUNDS
    assert Tp % 2 == 0 and 4 * Tp * 32 < 2**16

    nc = bacc.Bacc("TRN2", target_bir_lowering=False, debug=False, num_devices=NCORES)

    xyt_d = nc.dram_tensor("xyt", [P, 2, NE], f32, kind="ExternalInput")
    zzt_d = nc.dram_tensor("zzt", [P, 2, NE], f32, kind="ExternalInput")
    kix_d = nc.dram_tensor("kix", [P, 4 * NE], i16, kind="ExternalInput")
    jix_d = nc.dram_tensor("jix", [P, 4 * NE], i16, kind="ExternalInput")
    pix_d = nc.dram_tensor("pix", [P, 4 * Tp], i16, kind="ExternalInput")
    scal_d = nc.dram_tensor("scal", [P, 3], f32, kind="ExternalInput")
    clo_d = nc.dram_tensor("clo", [P, 32], f32, kind="ExternalInput")
    chi_d = nc.dram_tensor("chi", [P, 32], f32, kind="ExternalInput")
    out_d = nc.dram_tensor("out", [P, 64], f32, kind="ExternalOutput")

    with tile.TileContext(nc) as tc:
        with tc.tile_pool(name="main", bufs=1) as pool:
            XYT = pool.tile([P, 2, NE], f32)
            nc.sync.dma_start(XYT, xyt_d.ap())
            ZZT = pool.tile([P, 2, NE], f32)
            nc.sync.dma_start(ZZT, zzt_d.ap())
            KIX = pool.tile([P, 4 * NE], i16)
            nc.sync.dma_start(KIX, kix_d.ap())
            JIX = pool.tile([P, 4 * NE], i16)
            nc.sync.dma_start(JIX, jix_d.ap())
            PIX = pool.tile([P, 4 * Tp], i16)
            nc.sync.dma_start(PIX, pix_d.ap())
            SCAL = pool.tile([P, 3], f32)
            nc.sync.dma_start(SCAL, scal_d.ap())
            CLO = pool.tile([P, 32], f32)
            nc.sync.dma_start(CLO, clo_d.ap())
            CHI = pool.tile([P, 32], f32)
            nc.sync.dma_start(CHI, chi_d.ap())

            EPS = pool.tile([P, 1], f32)
            nc.vector.memset(EPS, 1e-12)
            HPI = pool.tile([P, 1], f32)
            nc.vector.memset(HPI, PI / 2.0)

            # side tiles: fields x, y, z, zn
            KF = pool.tile([P, 4, Tp], f32)   # k side, canonical order
            JS = pool.tile([P, 4, Tp], f32)   # j side, j-sorted order
            FJ = pool.tile([P, 4, Tp], f32)   # j side, canonical order

            def scat(out2, data2, idx, nidx):
                nc.gpsimd.local_scatter(
                    out2.bitcast(i16), data2.bitcast(i16), idx,
                    channels=P, num_elems=4 * Tp, num_idxs=nidx,
                )

            # j side first: its chain (scatter -> fill -> perm) is the longest
            scat(JS[:, 0:2], XYT, JIX, 4 * NE)
            scat(JS[:, 2:4], ZZT, JIX, 4 * NE)
            scat(KF[:, 0:2], XYT, KIX, 4 * NE)
            scat(KF[:, 2:4], ZZT, KIX, 4 * NE)

            # jump-fill duplicates (runs marked by x == 0 gaps), bundled over
            # the 4 fields with a stride-0 broadcast of the gap mask
            M = pool.tile([P, Tp], f32)
            TMP = pool.tile([P, 4, Tp], f32)

            def fill(SIDE):
                for r in range(rounds):
                    d = 1 << r
                    nc.vector.tensor_scalar(out=M[:, d:Tp], in0=SIDE[:, 0, d:Tp], scalar1=0.0, scalar2=None, op0=alu.is_equal)
                    Mb = M[:, d:Tp].rearrange("p (a t) -> p a t", a=1).broadcast_to([P, 4, Tp - d])
                    nc.vector.tensor_tensor(out=TMP[:, :, d:Tp], in0=SIDE[:, :, 0 : Tp - d], in1=Mb, op=alu.mult)
                    nc.vector.tensor_tensor(out=SIDE[:, :, d:Tp], in0=SIDE[:, :, d:Tp], in1=TMP[:, :, d:Tp], op=alu.add)

            fill(JS)

            # permute j side into canonical order (exact f32 bit moves)
            scat(FJ[:, 0:2], JS[:, 0:2], PIX, 4 * Tp)
            scat(FJ[:, 2:4], JS[:, 2:4], PIX, 4 * Tp)

            fill(KF)

            # ---- geometry ----
            D = pool.tile([P, 3, 2, Tp], f32)   # [coord, side(j,k), slot]
            R2 = pool.tile([P, 3, Tp], f32)     # [rij2, rik2, rjk2] pre-sqrt
            t1 = pool.tile([P, Tp], f32)
            t2 = pool.tile([P, Tp], f32)

            def side_geom(SRC, s, r2out):
                # D[:, q, s] = SRC[:, q] - p_i;  r2out = |D|^2
                for q in range(3):
                    vi = SCAL[:, q : q + 1]
                    nc.vector.tensor_scalar(out=D[:, q, s], in0=SRC[:, q], scalar1=vi, scalar2=None, op0=alu.subtract)
                nc.vector.tensor_tensor(out=t1, in0=D[:, 0, s], in1=D[:, 0, s], op=alu.mult)
                nc.vector.tensor_tensor(out=t2, in0=D[:, 1, s], in1=D[:, 1, s], op=alu.mult)
                nc.vector.tensor_tensor(out=t1, in0=t1, in1=t2, op=alu.add)
                nc.vector.tensor_tensor(out=t2, in0=D[:, 2, s], in1=D[:, 2, s], op=alu.mult)
                nc.vector.tensor_tensor(out=r2out, in0=t1, in1=t2, op=alu.add)

            side_geom(KF, 1, R2[:, 1])
            side_geom(FJ, 0, R2[:, 0])

            u1 = pool.tile([P, Tp], f32)
            u2 = pool.tile([P, Tp], f32)
            u3 = pool.tile([P, Tp], f32)
            nc.vector.tensor_tensor(out=u1, in0=D[:, 0, 0], in1=D[:, 0, 1], op=alu.subtract)
            nc.vector.tensor_tensor(out=u2, in0=u1, in1=u1, op=alu.mult)
            nc.vector.tensor_tensor(out=u1, in0=D[:, 1, 0], in1=D[:, 1, 1], op=alu.subtract)
            nc.vector.tensor_tensor(out=u3, in0=u1, in1=u1, op=alu.mult)
            nc.vector.tensor_tensor(out=u2, in0=u2, in1=u3, op=alu.add)
            nc.vector.tensor_tensor(out=u1, in0=D[:, 2, 0], in1=D[:, 2, 1], op=alu.subtract)
            nc.vector.tensor_tensor(out=u3, in0=u1, in1=u1, op=alu.mult)
            nc.vector.tensor_tensor(out=R2[:, 2], in0=u2, in1=u3, op=alu.add)

            # r = sqrt(r2 + 1e-12); downstream uses r*r (reference quirk)
            R = pool.tile([P, 3, Tp], f32)
            Rf = R.rearrange("p a s -> p (a s)")
            nc.scalar.activation(Rf, R2.rearrange("p a s -> p (a s)"), AF.Sqrt, bias=EPS)
            SQ2 = pool.tile([P, 3, Tp], f32)
            nc.vector.tensor_tensor(out=SQ2.rearrange("p a s -> p (a s)"), in0=Rf, in1=Rf, op=alu.mult)
            a2, b2, c2 = SQ2[:, 0], SQ2[:, 1], SQ2[:, 2]

            sab = pool.tile([P, Tp], f32)
            nc.vector.tensor_tensor(out=sab, in0=a2, in1=b2, op=alu.add)
            S3F = pool.tile([P, Tp], f32)
            nc.vector.tensor_tensor(out=S3F, in0=sab, in1=c2, op=alu.add)
            NUM = pool.tile([P, Tp], f32)
            nc.vector.tensor_tensor(out=NUM, in0=sab, in1=c2, op=alu.subtract)

            # cutoff: fc(r) = cos(pi r / 12)^2 * (r2 < 36)
            RCL = pool.tile([P, 3, Tp], f32)
            nc.vector.tensor_scalar(out=RCL.rearrange("p a s -> p (a s)"), in0=Rf, scalar1=CUTOFF, scalar2=None, op0=alu.min)
            CS = pool.tile([P, 3, Tp], f32)
            nc.scalar.activation(
                CS.rearrange("p a s -> p (a s)"), RCL.rearrange("p a s -> p (a s)"),
                AF.Sin, scale=PI / 12.0, bias=HPI,
            )
            nc.vector.tensor_tensor(out=u1, in0=CS[:, 0], in1=CS[:, 1], op=alu.mult)
            nc.vector.tensor_tensor(out=u1, in0=u1, in1=CS[:, 2], op=alu.mult)
            CSQ = pool.tile([P, Tp], f32)
            nc.scalar.activation(CSQ, u1, AF.Square)

            nc.vector.tensor_tensor(out=u2, in0=a2, in1=b2, op=alu.max)
            nc.vector.tensor_tensor(out=u2, in0=u2, in1=c2, op=alu.max)
            nc.vector.tensor_scalar(out=u2, in0=u2, scalar1=CUTOFF * CUTOFF, scalar2=None, op0=alu.is_lt)

            # BASE = csq * ltmask * (znj * znk), clamped for Ln
            nc.vector.tensor_tensor(out=u3, in0=FJ[:, 3], in1=KF[:, 3], op=alu.mult)
            nc.vector.tensor_tensor(out=u2, in0=u2, in1=u3, op=alu.mult)
            nc.vector.tensor_tensor(out=u2, in0=u2, in1=CSQ, op=alu.mult)
            nc.vector.tensor_scalar(out=u2, in0=u2, scalar1=1e-30, scalar2=None, op0=alu.max)
            LNBF = pool.tile([P, Tp], f32)
            nc.scalar.activation(LNBF, u2, AF.Ln)

            # ln(1 - cos theta) = ln(2*rij*rik - NUM) - ln(2*rij*rik)
            RR = pool.tile([P, Tp], f32)
            nc.vector.tensor_tensor(out=RR, in0=R[:, 0], in1=R[:, 1], op=alu.mult)
            nc.vector.scalar_tensor_tensor(out=u1, in0=RR, scalar=2.0, in1=NUM, op0=alu.mult, op1=alu.subtract)
            nc.vector.tensor_scalar(out=u1, in0=u1, scalar1=1e-30, scalar2=None, op0=alu.max)
            nc.scalar.activation(u2, u1, AF.Ln)
            nc.scalar.activation(u3, RR, AF.Ln, scale=2.0)
            LNCF = pool.tile([P, Tp], f32)
            nc.vector.tensor_tensor(out=LNCF, in0=u2, in1=u3, op=alu.subtract)

            # ---- pair stage ----
            LNU = pool.tile([P, 4, Tp], f32)
            for z in range(4):
                nc.vector.scalar_tensor_tensor(
                    out=LNU[:, z], in0=LNCF, scalar=float(ZETAS[z]), in1=LNBF,
                    op0=alu.mult, op1=alu.add,
                )

            S3b = S3F.rearrange("p (a t) -> p a t", a=1).broadcast_to([P, 4, Tp])
            PART = pool.tile([P, 32], f32)
            XA = pool.tile([P, 4, Tp], f32)
            XB = pool.tile([P, 4, Tp], f32)
            EA = pool.tile([P, 4, Tp], f32)
            EB = pool.tile([P, 4, Tp], f32)
            for e in range(8):
                X = XA if e % 2 == 0 else XB
                E = EA if e % 2 == 0 else EB
                nc.vector.scalar_tensor_tensor(
                    out=X, in0=S3b, scalar=float(-etas[e]), in1=LNU,
                    op0=alu.mult, op1=alu.add,
                )
                nc.scalar.activation(
                    E.rearrange("p z t -> p (z t)"),
                    X.rearrange("p z t -> p (z t)"), AF.Exp,
                )
                nc.vector.tensor_reduce(
                    out=PART[:, e * 4 : (e + 1) * 4], in_=E,
                    axis=mybir.AxisListType.X, op=alu.add,
                )

            # ---- final scaling into [128, 64] ----
            OUT = pool.tile([P, 64], f32)
            Ov = OUT.rearrange("p (e g z) -> p e g z", e=8, g=2, z=4)
            Pv = PART.rearrange("p (e z) -> p e z", e=8, z=4)
            Lv = CLO.rearrange("p (e z) -> p e z", e=8, z=4)
            Hv = CHI.rearrange("p (e z) -> p e z", e=8, z=4)
            nc.vector.tensor_tensor(out=Ov[:, :, 0], in0=Pv, in1=Lv, op=alu.mult)
            nc.vector.tensor_tensor(out=Ov[:, :, 1], in0=Pv, in1=Hv, op=alu.mult)
            nc.sync.dma_start(out_d.ap(), OUT)

    nc.compile()
    return nc


def _first_occurrence_slots(sorted_vals: np.ndarray) -> np.ndarray:
    """sorted_vals [P, Tp] ascending. Returns [P, NE] int16: first slot of
    each entry value, -1 if absent."""
    Pn, Tpn = sorted_vals.shape
    fm = np.ones((Pn, Tpn), dtype=bool)
    fm[:, 1:] = sorted_vals[:, 1:] != sorted_vals[:, :-1]
    idx = np.full((Pn, NE), -1, np.int64)
    pp, ss = np.nonzero(fm)
    idx[pp, sorted_vals[pp, ss]] = ss
    return idx


def _prepare_host(inputs):
    global _ROUNDS
    positions = np.asarray(inputs["positions"], dtype=np.float32)
    nj = np.asarray(inputs["neighbors_j"]).astype(np.int64)
    nk = np.asarray(inputs["neighbors_k"]).astype(np.int64)
    mask = np.asarray(inputs["mask_triples"]) != 0
    atomic = np.asarray(inputs["atomic_numbers"]).astype(np.float32)
    etas = np.asarray(inputs["etas"], dtype=np.float32)

    counts = mask.sum(axis=2)
    Tp = int(counts.max())
    Tp = max(16, ((Tp + 15) // 16) * 16)
    assert 4 * Tp * 32 < 2**16, f"Tp={Tp} too large for local_scatter"

    order = np.argsort(~mask, axis=2, kind="stable")
    valid = np.take_along_axis(mask, order, 2)[:, :, :Tp]
    pad_ord = np.cumsum(~valid, axis=2) - 1
    dummy_id = 256 + pad_ord % NDUM
    jpad = np.where(valid, np.take_along_axis(nj, order, 2)[:, :, :Tp], dummy_id)
    kpad = np.where(valid, np.take_along_axis(nk, order, 2)[:, :, :Tp], dummy_id)

    clo_row = np.array([2.0 ** (1.0 - z) for _ in range(8) for z in ZETAS], dtype=np.float32)
    chi_row = np.array([2.0 ** (1.0 + z) for _ in range(8) for z in ZETAS], dtype=np.float32)
    clo = np.broadcast_to(clo_row, (P, 32)).copy()
    chi = np.broadcast_to(chi_row, (P, 32)).copy()

    # interleaved lo/hi scatter index arrays: input element i = f*2*NE + 2e + h
    # (int16 view of data [P, 2, NE] f32) -> dst int16 pos = f*2*Tp + 2*slot + h
    e_h = np.arange(2)  # lo/hi
    maxrun = 1
    in_maps = []
    for c in range(NCORES):
        b, h = divmod(c, 2)
        asl = slice(h * P, (h + 1) * P)
        jp = jpad[b, asl]    # [P, Tp]
        kp = kpad[b, asl]

        korder = np.argsort(kp, axis=1, kind="stable")
        kcan = np.take_along_axis(kp, korder, 1)
        jcan = np.take_along_axis(jp, korder, 1)
        jorder = np.argsort(jcan, axis=1, kind="stable")
        jsorted = np.take_along_axis(jcan, jorder, 1)

        kidx = _first_occurrence_slots(kcan)   # [P, NE]
        jidx = _first_occurrence_slots(jsorted)

        # max run length (for fill rounds)
        for arr in (kcan, jsorted):
            fm = np.ones_like(arr, dtype=bool)
            fm[:, 1:] = arr[:, 1:] != arr[:, :-1]
            runpos = np.nonzero(fm.ravel())[0]
            runlen = np.diff(np.append(runpos, arr.size))
            maxrun = max(maxrun, int(runlen.max()))

        def table_idx(idx258):
            # [P, 2, NE, 2]: f, e, h -> f*2*Tp + 2*slot + h (or -1)
            s = idx258[:, None, :, None]
            f = np.arange(2)[None, :, None, None]
            hh = e_h[None, None, None, :]
            arr = np.where(s >= 0, f * 2 * Tp + 2 * s + hh, -1)
            return arr.reshape(P, 4 * NE).astype(np.int16)

        kix = table_idx(kidx)
        jix = table_idx(jidx)

        # perm: input element i = f*2*Tp + 2t + h -> dst f*2*Tp + 2*jorder[t] + h
        s = jorder[:, None, :, None]
        f = np.arange(2)[None, :, None, None]
        hh = e_h[None, None, None, :]
        pix = (f * 2 * Tp + 2 * s + hh).reshape(P, 4 * Tp).astype(np.int16)

        # shared tables (replicated across partitions)
        fars = np.full(NDUM, FAR, np.float32)
        xrow = np.concatenate([positions[b, :, 0], fars]).astype(np.float32)
        yrow = np.concatenate([positions[b, :, 1], fars]).astype(np.float32)
        zrow = np.concatenate([positions[b, :, 2], fars]).astype(np.float32)
        znrow = np.concatenate([atomic[b], np.zeros(NDUM, np.float32)]).astype(np.float32)
        xyt = np.broadcast_to(np.stack([xrow, yrow]), (P, 2, NE)).copy()
        zzt = np.broadcast_to(np.stack([zrow, znrow]), (P, 2, NE)).copy()

        scal = positions[b, asl].copy()
        in_maps.append({
            "xyt": xyt, "zzt": zzt, "kix": kix, "jix": jix, "pix": pix,
            "scal": scal, "clo": clo, "chi": chi,
        })

    _ROUNDS = max(1, int(np.ceil(np.log2(max(2, maxrun)))))
    return Tp, etas, in_maps


def kernel(**inputs) -> np.ndarray:
    Tp, etas, in_maps = _prepare_host(inputs)
    nc = _build_program(Tp, etas)
    res = run_bass_kernel_spmd(nc, in_maps, core_ids=list(range(NCORES)))
    out = np.zeros((B, A, 64), np.float32)
    for c in range(NCORES):
        b, h = divmod(c, 2)
        out[b, h * P : (h + 1) * P] = res.results[c]["out"]
    return out



# revision 4
# speedup vs baseline: 1.2641x; 1.2641x over previous
"""Trainium2 Bass kernel for nn_BehlerG2 (Behler-style angular symmetry functions).

Strategy (v4 — scan-fill + factorized pair stage):
- 8 cores; core c handles batch b = c // 2, atom half h = c % 2 (128 atoms/core,
  one atom per SBUF partition).
- Host compacts each atom's triple list by mask, pads to Tp with dummy entry 256
  (coords 1e4, beyond cutoff -> ~0 contribution; z_ijk = 0 kills it exactly).
- Canonical slot order is k-sorted. Neighbor coords (x,y,z) are materialized with
  3 merged GPSIMD local_scatters (3 f32 fields = 6 int16 per entry each):
  k-table -> KF, j-table -> JS (j-sorted), JS -> FJ (perm into canonical).
  Run expansion uses ONE tensor_tensor_scan per coord plane:
      state = gapmask[t]*state + val[t]   (op0=mult, op1=add)
  so run length is unbounded (no log-rounds fill).
- z_ijk = zn_j*zn_k is integer-derived metadata; host computes it exactly in f32
  and DMAs it per-slot (kills 2 scatters of the old design).
- Math: zetas {1,2,4,8} are integer powers -> repeated squaring (no Ln/log-space).
  exp factorizes: E[e,z,t] = U_e[t]*H_z[t] with U_e = exp(-eta_e*S3),
  H_z = W*base^z, W = (fc_ij*fc_ik*fc_jk)^2... * z_ijk.  cutoff handled by
  min(r,6) -> sin(pi)≈0; garbage cos for dead triples bounded by clamping
  base1 to [-2,2] (valid triples always lie in [0,2]).
- Pair reductions: 32 DVE tensor_tensor_reduce ops (fused product+sum),
  U/H planes in fp16 (2x DVE mode eligible; positive sums average the rounding).
  Per-z fp16-overflow scales (1,1,4,64) are folded into the output weights.
- All heavy elementwise work split across DVE / Pool / ACT engines.
"""

import sys

if "/opt/trn_rl_repo" not in sys.path:
    sys.path.insert(0, "/opt/trn_rl_repo")

import numpy as np

import concourse.bacc as bacc
import concourse.mybir as mybir
import concourse.tile as tile
from concourse.alu_op_type import AluOpType as alu
from concourse.bass_utils import run_bass_kernel_spmd

f32 = mybir.dt.float32
f16 = mybir.dt.float16
i16 = mybir.dt.int16

B, A, T = 4, 256, 512
NCORES = 8
P = 128          # atoms per core == partitions
NE = 258         # table entries: 256 atoms + dummy(256) + spare(257)
ZETAS = [1.0, 2.0, 4.0, 8.0]
ZSC = [1.0, 1.0, 4.0, 64.0]   # fp16-overflow scale folded out of H4/H8
CUTOFF = 6.0
PI = float(np.pi)
FAR = 1.0e4      # dummy-entry coordinate (beyond cutoff)

AF = mybir.ActivationFunctionType


def _build_program(Tp: int, etas: np.ndarray):
    assert Tp % 2 == 0 and 6 * Tp * 32 < 2**16, f"Tp={Tp}"

    nc = bacc.Bacc("TRN2", target_bir_lowering=False, debug=False, num_devices=NCORES)

    tab_d = nc.dram_tensor("tab", [P, 3, NE], f32, kind="ExternalInput")
    kix_d = nc.dram_tensor("kix", [P, 6 * NE], i16, kind="ExternalInput")
    jix_d = nc.dram_tensor("jix", [P, 6 * NE], i16, kind="ExternalInput")
    pix_d = nc.dram_tensor("pix", [P, 6 * Tp], i16, kind="ExternalInput")
    scal_d = nc.dram_tensor("scal", [P, 3], f32, kind="ExternalInput")
    zijk_d = nc.dram_tensor("zijk", [P, Tp], f32, kind="ExternalInput")
    clo_d = nc.dram_tensor("clo", [P, 32], f32, kind="ExternalInput")
    chi_d = nc.dram_tensor("chi", [P, 32], f32, kind="ExternalInput")
    out_d = nc.dram_tensor("out", [P, 64], f32, kind="ExternalOutput")

    with tile.TileContext(nc) as tc:
        with tc.tile_pool(name="main", bufs=1) as pool:
            TAB = pool.tile([P, 3, NE], f32)
            nc.sync.dma_start(TAB, tab_d.ap())
            KIX = pool.tile([P, 6 * NE], i16)
            nc.sync.dma_start(KIX, kix_d.ap())
            JIX = pool.tile([P, 6 * NE], i16)
            nc.sync.dma_start(JIX, jix_d.ap())
            PIX = pool.tile([P, 6 * Tp], i16)
            nc.sync.dma_start(PIX, pix_d.ap())
            SCAL = pool.tile([P, 3], f32)
            nc.sync.dma_start(SCAL, scal_d.ap())
            ZIJK = pool.tile([P, Tp], f32)
            nc.sync.dma_start(ZIJK, zijk_d.ap())
            CLO = pool.tile([P, 32], f32)
            nc.sync.dma_start(CLO, clo_d.ap())
            CHI = pool.tile([P, 32], f32)
            nc.sync.dma_start(CHI, chi_d.ap())

            EPS = pool.tile([P, 1], f32)
            nc.vector.memset(EPS, 1e-12)
            HPI = pool.tile([P, 1], f32)
            nc.vector.memset(HPI, PI / 2.0)
            TINY = pool.tile([P, 2], f32)

            # preload the sqrt activation table while scatters run
            nc.scalar.activation(TINY, EPS.broadcast_to([P, 2]), AF.Sqrt)

            # side tiles: SIDES[:,0] = j side (FJ), SIDES[:,1] = k side (KF)
            SIDES = pool.tile([P, 2, 3, Tp], f32)
            FJ = SIDES[:, 0]
            KF = SIDES[:, 1]
            JS = pool.tile([P, 3, Tp], f32)   # j side, j-sorted order

            def scat(out3, data3, idx, nidx):
                nc.gpsimd.local_scatter(
                    out3.bitcast(i16).rearrange("p a b -> p (a b)"),
                    data3.bitcast(i16).rearrange("p a b -> p (a b)"),
                    idx, channels=P, num_elems=6 * Tp, num_idxs=nidx,
                )

            M = pool.tile([P, Tp], f32)

            def fill(SIDE):
                # gap mask from the x plane (scatter zeroes gaps; no real
                # coordinate nor FAR is ever 0.0)
                nc.vector.tensor_scalar(out=M, in0=SIDE[:, 0], scalar1=0.0,
                                        scalar2=None, op0=alu.is_equal)
                for q in range(3):
                    nc.vector.tensor_tensor_scan(
                        out=SIDE[:, q], data0=M, data1=SIDE[:, q],
                        initial=0.0, op0=alu.mult, op1=alu.add,
                    )

            SCB = SCAL.rearrange("p (c t) -> p c t", t=1).broadcast_to([P, 3, Tp])

            # ---- front-end ----
            scat(KF, TAB, KIX, 6 * NE)
            fill(KF)
            scat(JS, TAB, JIX, 6 * NE)

            # k-side geometry on V while scat_j runs
            DSQ = pool.tile([P, 3, Tp], f32)
            R2 = pool.tile([P, 3, Tp], f32)   # [rij2, rik2, rjk2]
            TMP = pool.tile([P, Tp], f32)
            nc.vector.tensor_tensor(out=KF, in0=KF, in1=SCB, op=alu.subtract)  # Dk
            nc.vector.tensor_tensor(out=DSQ, in0=KF, in1=KF, op=alu.mult)
            nc.vector.tensor_tensor(out=TMP, in0=DSQ[:, 0], in1=DSQ[:, 1], op=alu.add)
            nc.vector.tensor_tensor(out=R2[:, 1], in0=TMP, in1=DSQ[:, 2], op=alu.add)

            fill(JS)
            scat(FJ, JS, PIX, 6 * Tp)

            # j side + cross terms; U-chain on Pool, j-chain on V
            nc.vector.tensor_tensor(out=FJ, in0=FJ, in1=SCB, op=alu.subtract)  # Dj
            U3 = pool.tile([P, 3, Tp], f32)
            PT = pool.tile([P, Tp], f32)
            nc.gpsimd.tensor_tensor(out=U3, in0=FJ, in1=KF, op=alu.subtract)
            nc.gpsimd.tensor_tensor(out=U3, in0=U3, in1=U3, op=alu.mult)
            nc.gpsimd.tensor_tensor(out=PT, in0=U3[:, 0], in1=U3[:, 1], op=alu.add)
            nc.gpsimd.tensor_tensor(out=R2[:, 2], in0=PT, in1=U3[:, 2], op=alu.add)

            nc.vector.tensor_tensor(out=DSQ, in0=FJ, in1=FJ, op=alu.mult)
            nc.vector.tensor_tensor(out=TMP, in0=DSQ[:, 0], in1=DSQ[:, 1], op=alu.add)
            nc.vector.tensor_tensor(out=R2[:, 0], in0=TMP, in1=DSQ[:, 2], op=alu.add)

            # ---- scalar geometry ----
            R = pool.tile([P, 3, Tp], f32)
            Rf = R.rearrange("p a s -> p (a s)")
            nc.scalar.activation(Rf, R2.rearrange("p a s -> p (a s)"), AF.Sqrt, bias=EPS)

            RR = pool.tile([P, Tp], f32)
            RCP = pool.tile([P, Tp], f32)
            nc.vector.tensor_tensor(out=RR, in0=R[:, 0], in1=R[:, 1], op=alu.mult)
            nc.vector.reciprocal(RCP, RR)
            # clamp r to cutoff for the cosine window (R raw no longer needed)
            nc.vector.tensor_scalar(out=Rf, in0=Rf, scalar1=CUTOFF, scalar2=None, op0=alu.min)
            CS = pool.tile([P, 3, Tp], f32)
            nc.scalar.activation(CS.rearrange("p a s -> p (a s)"), Rf,
                                 AF.Sin, scale=PI / 12.0, bias=HPI)

            SP = pool.tile([P, Tp], f32)
            S3 = pool.tile([P, Tp], f32)
            NUM = pool.tile([P, Tp], f32)
            nc.vector.tensor_tensor(out=SP, in0=R2[:, 0], in1=R2[:, 1], op=alu.add)
            nc.vector.tensor_tensor(out=S3, in0=SP, in1=R2[:, 2], op=alu.add)
            nc.vector.tensor_tensor(out=NUM, in0=SP, in1=R2[:, 2], op=alu.subtract)

            # base1 = 1 - cos(theta) = (RR - 0.5*NUM) / RR, clamped to [-2, 2]
            B1 = pool.tile([P, Tp], f32)
            nc.vector.scalar_tensor_tensor(out=B1, in0=NUM, scalar=-0.5, in1=RR,
                                           op0=alu.mult, op1=alu.add)
            nc.vector.tensor_tensor(out=B1, in0=B1, in1=RCP, op=alu.mult)
            nc.vector.tensor_scalar(out=B1, in0=B1, scalar1=2.0, scalar2=-2.0,
                                    op0=alu.min, op1=alu.max)

            # W = (cs_ij*cs_ik*cs_jk)^2 * z_ijk  (Pool)
            CP = pool.tile([P, Tp], f32)
            W = pool.tile([P, Tp], f32)
            nc.gpsimd.tensor_tensor(out=CP, in0=CS[:, 0], in1=CS[:, 1], op=alu.mult)
            nc.gpsimd.tensor_tensor(out=CP, in0=CP, in1=CS[:, 2], op=alu.mult)
            nc.gpsimd.tensor_tensor(out=CP, in0=CP, in1=CP, op=alu.mult)
            nc.gpsimd.tensor_tensor(out=W, in0=CP, in1=ZIJK, op=alu.mult)

            # U_e = exp(-eta_e * S3), fp16 (exp table load happens once here)
            UE = pool.tile([P, 8, Tp], f16)
            for e in range(8):
                nc.scalar.activation(UE[:, e], S3, AF.Exp, scale=float(-etas[e]))

            # base powers (with fp16-overflow scales) and H planes
            B2 = pool.tile([P, Tp], f32)
            B4 = pool.tile([P, Tp], f32)
            B8 = pool.tile([P, Tp], f32)
            nc.vector.tensor_tensor(out=B2, in0=B1, in1=B1, op=alu.mult)
            nc.vector.scalar_tensor_tensor(out=B4, in0=B2, scalar=0.25, in1=B2,
                                           op0=alu.mult, op1=alu.mult)
            nc.vector.scalar_tensor_tensor(out=B8, in0=B4, scalar=0.25, in1=B4,
                                           op0=alu.mult, op1=alu.mult)
            H = pool.tile([P, 4, Tp], f16)
            for z, BP in enumerate((B1, B2, B4, B8)):
                nc.vector.tensor_tensor(out=H[:, z], in0=W, in1=BP, op=alu.mult)

            # ---- pair stage: PART[e,z] = sum_t U_e * H_z ----
            PART = pool.tile([P, 32], f32)
            SCR0 = pool.tile([P, Tp], f16)
            SCR1 = pool.tile([P, Tp], f16)
            SCR = [SCR0, SCR1]
            for e in range(8):
                for z in range(4):
                    idx = e * 4 + z
                    nc.vector.scalar_tensor_tensor(
                        out=SCR[idx % 2], in0=UE[:, e], scalar=1.0, in1=H[:, z],
                        op0=alu.mult, op1=alu.mult,
                        accum_out=PART[:, idx : idx + 1],
                    )

            # ---- final scaling into [128, 64] ----
            OUT = pool.tile([P, 64], f32)
            Ov = OUT.rearrange("p (e g z) -> p e g z", e=8, g=2, z=4)
            Pv = PART.rearrange("p (e z) -> p e z", e=8, z=4)
            Lv = CLO.rearrange("p (e z) -> p e z", e=8, z=4)
            Hv = CHI.rearrange("p (e z) -> p e z", e=8, z=4)
            nc.vector.tensor_tensor(out=Ov[:, :, 0], in0=Pv, in1=Lv, op=alu.mult)
            nc.vector.tensor_tensor(out=Ov[:, :, 1], in0=Pv, in1=Hv, op=alu.mult)
            nc.sync.dma_start(out_d.ap(), OUT)

    nc.compile()
    return nc


def _first_occurrence_slots(sorted_vals: np.ndarray) -> np.ndarray:
    """sorted_vals [P, Tp] ascending. Returns [P, 257] int64: first slot of
    each entry value, -1 if absent."""
    Pn, Tpn = sorted_vals.shape
    fm = np.ones((Pn, Tpn), dtype=bool)
    fm[:, 1:] = sorted_vals[:, 1:] != sorted_vals[:, :-1]
    idx = np.full((Pn, NE), -1, np.int64)
    pp, ss = np.nonzero(fm)
    idx[pp, sorted_vals[pp, ss]] = ss
    return idx


def _table_idx(idx_slots: np.ndarray, Tp: int) -> np.ndarray:
    """idx_slots [P, NE] (slot or -1) -> int16 [P, 6*NE]: for data element
    i = f*2*NE + 2*e + h, destination = f*2*Tp + 2*slot[e] + h (or -1)."""
    s = idx_slots[:, None, :, None]                      # [P,1,NE,1]
    f = np.arange(3)[None, :, None, None]
    hh = np.arange(2)[None, None, None, :]
    arr = np.where(s >= 0, f * 2 * Tp + 2 * s + hh, -1)
    return arr.reshape(P, 6 * NE).astype(np.int16)


def _prepare_host(inputs):
    positions = np.asarray(inputs["positions"], dtype=np.float32)
    nj = np.asarray(inputs["neighbors_j"]).astype(np.int64)
    nk = np.asarray(inputs["neighbors_k"]).astype(np.int64)
    mask = np.asarray(inputs["mask_triples"]) != 0
    atomic = np.asarray(inputs["atomic_numbers"]).astype(np.float32)
    etas = np.asarray(inputs["etas"], dtype=np.float32)

    assert not np.any(positions == 0.0), "scan gap-mask relies on nonzero coords"

    counts = mask.sum(axis=2)
    Tp = int(counts.max())
    Tp = max(16, ((Tp + 15) // 16) * 16)
    assert 6 * Tp * 32 < 2**16, f"Tp={Tp} too large for merged local_scatter"

    order = np.argsort(~mask, axis=2, kind="stable")
    valid = np.take_along_axis(mask, order, 2)[:, :, :Tp]
    jpad = np.where(valid, np.take_along_axis(nj, order, 2)[:, :, :Tp], 256)
    kpad = np.where(valid, np.take_along_axis(nk, order, 2)[:, :, :Tp], 256)

    clo_row = np.array([(2.0 ** (1.0 - zv)) * sc for _ in range(8)
                        for zv, sc in zip(ZETAS, ZSC)], dtype=np.float32)
    chi_row = np.array([(2.0 ** (1.0 + zv)) * sc for _ in range(8)
                        for zv, sc in zip(ZETAS, ZSC)], dtype=np.float32)
    clo = np.broadcast_to(clo_row, (P, 32)).copy()
    chi = np.broadcast_to(chi_row, (P, 32)).copy()

    in_maps = []
    for c in range(NCORES):
        b, h = divmod(c, 2)
        asl = slice(h * P, (h + 1) * P)
        jp = jpad[b, asl]    # [P, Tp]
        kp = kpad[b, asl]

        korder = np.argsort(kp, axis=1, kind="stable")
        kcan = np.take_along_axis(kp, korder, 1)
        jcan = np.take_along_axis(jp, korder, 1)
        jorder = np.argsort(jcan, axis=1, kind="stable")
        jsorted = np.take_along_axis(jcan, jorder, 1)

        kix = _table_idx(_first_occurrence_slots(kcan), Tp)
        jix = _table_idx(_first_occurrence_slots(jsorted), Tp)

        # perm: data element i = f*2*Tp + 2t + h -> dst f*2*Tp + 2*jorder[t] + h
        s = jorder[:, None, :, None]
        f = np.arange(3)[None, :, None, None]
        hh = np.arange(2)[None, None, None, :]
        pix = (f * 2 * Tp + 2 * s + hh).reshape(P, 6 * Tp).astype(np.int16)

        # shared coordinate table (replicated across partitions)
        fars = np.full(NE - 256, FAR, np.float32)
        tab = np.stack([
            np.concatenate([positions[b, :, q], fars]) for q in range(3)
        ]).astype(np.float32)
        tab = np.broadcast_to(tab, (P, 3, NE)).copy()

        # z_ijk per canonical slot (dummy entries -> 0)
        z258 = np.concatenate([atomic[b], np.zeros(NE - 256, np.float32)])
        zijk = (z258[jcan] * z258[kcan]).astype(np.float32)

        scal = positions[b, asl].copy()
        in_maps.append({
            "tab": tab, "kix": kix, "jix": jix, "pix": pix,
            "scal": scal, "zijk": zijk, "clo": clo, "chi": chi,
        })

    return Tp, etas, in_maps


def kernel(**inputs) -> np.ndarray:
    Tp, etas, in_maps = _prepare_host(inputs)
    nc = _build_program(Tp, etas)
    res = run_bass_kernel_spmd(nc, in_maps, core_ids=list(range(NCORES)))
    out = np.zeros((B, A, 64), np.float32)
    for c in range(NCORES):
        b, h = divmod(c, 2)
        out[b, h * P : (h + 1) * P] = res.results[c]["out"]
    return out
